# revision 6
# baseline (speedup 1.0000x reference)
"""ClusterAwareBatchNorm2d on 8 Trainium2 NeuronCores.

Strategy (batch-sharded, single kernel launch):
  - Each core owns 8 of the 64 samples (contiguous slab of x).
  - Pass 1 (t-major order): stream the core's x shard through SBUF,
    computing per-(b,c) sum (DVE reduce) and sum-of-squares (ACT Square
    with accum_out). 10 of the 16 tiles stay resident in SBUF for pass 2.
  - TWO tiny AllGathers (8 KB/rank each), one per channel-tile half, of
    the RAW per-sample sums: the first collective's rank-arrival
    handshake overlaps the second half of pass 1, and its post-processing
    (mean, [c,b] transpose, gram matmul half) overlaps pass 1 too.
  - Every core redundantly runs FINCH first-partition clustering on-chip:
    gram matrix via PE, 1-NN via masked row-max + is_equal, connected
    components via 6 boolean matrix squarings (reachability closure),
    then cluster mean/var in matrix form (M @ stats), folded into a
    per-(b,c) affine A*x + B.
  - A per-core one-hot selection matrix (host input) picks the core's own
    8 rows of A/B; PE transposes them to [c, b] per-partition scale/bias.
  - Pass 2 (streamed-first): normalize in place on the ACT engine;
    stores are issued from the ACT engine's own HWDGE queue so they never
    head-of-line-block the SP load queue; the 6 streamed tiles are
    prefetched into 4 deep buffers during the collective window.
"""

import numpy as np
from contextlib import ExitStack

import concourse.bass as bass
import concourse.bacc as bacc
import concourse.tile as tile
import concourse.mybir as mybir
from concourse import bass_utils
from concourse.bass_interp import get_hw_module

F32 = mybir.dt.float32
AF = mybir.ActivationFunctionType
ALU = mybir.AluOpType
AX = mybir.AxisListType

B, C, H, W = 64, 256, 56, 56
HW = H * W                      # 3136
NCORES = 8
BL = B // NCORES                # 8 samples per core
CT = C // 128                   # 2 channel tiles
NTILES = BL * CT                # 16 x-tiles of [128, HW] per core
NRES = 10                       # tiles kept resident in SBUF across passes
EPS = 1e-5
NEG = -1.0e30


def build_program(rate_: float):
    nc = bacc.Bacc(
        "TRN2",
        target_bir_lowering=False,
        debug=False,
        num_devices=NCORES,
    )

    x_d = nc.dram_tensor("x", [BL, CT, 128, HW], F32, kind="ExternalInput")
    vb_d = nc.dram_tensor("vb", [B, C], F32, kind="ExternalInput")
    mb_d = nc.dram_tensor("mb", [B, C], F32, kind="ExternalInput")
    wt_d = nc.dram_tensor("wt", [B, C], F32, kind="ExternalInput")
    bs_d = nc.dram_tensor("bs", [B, C], F32, kind="ExternalInput")
    sel_d = nc.dram_tensor("sel", [B, BL], F32, kind="ExternalInput")
    id_d = nc.dram_tensor("ident", [128, 128], F32, kind="ExternalInput")
    out_d = nc.dram_tensor("out", [BL, CT, 128, HW], F32, kind="ExternalOutput")

    # pass-1 order: t-major so channel-tile 0's stats complete halfway in
    p1_order = [(b, t) for t in range(CT) for b in range(BL)]
    idx_stream = p1_order[: NTILES - NRES]
    idx_res = p1_order[NTILES - NRES :]

    with tile.TileContext(nc, num_cores=NCORES) as tc, ExitStack() as ctx:
        sb = ctx.enter_context(tc.tile_pool(name="sb", bufs=1))
        res = ctx.enter_context(tc.tile_pool(name="res", bufs=NRES))
        xs = ctx.enter_context(tc.tile_pool(name="xs", bufs=3))
        ps = ctx.enter_context(tc.tile_pool(name="ps", bufs=2, space="PSUM"))
        ps1 = ctx.enter_context(tc.tile_pool(name="ps1", bufs=1, space="PSUM"))
        dram = ctx.enter_context(tc.tile_pool(name="dram", bufs=1, space="DRAM"))

        # small constants via SWDGE (keeps the SP HWDGE queue free for x)
        ident = sb.tile([128, 128], F32, tag="ident")
        nc.gpsimd.dma_start(out=ident, in_=id_d[:, :])
        sel_sb = sb.tile([B, BL], F32, tag="sel")
        nc.gpsimd.dma_start(out=sel_sb, in_=sel_d[:, :])
        vb_sb = sb.tile([B, C], F32, tag="vb")
        nc.gpsimd.dma_start(out=vb_sb, in_=vb_d[:, :])
        mb_sb = sb.tile([B, C], F32, tag="mb")
        nc.gpsimd.dma_start(out=mb_sb, in_=mb_d[:, :])
        wt_sb = sb.tile([B, C], F32, tag="wt")
        nc.gpsimd.dma_start(out=wt_sb, in_=wt_d[:, :])
        bs_sb = sb.tile([B, C], F32, tag="bs")
        nc.gpsimd.dma_start(out=bs_sb, in_=bs_d[:, :])

        # preload the ACT Sqrt table off the critical path
        sq_dummy = sb.tile([1, 1], F32, tag="sq_dummy")
        nc.scalar.sqrt(sq_dummy, ident[0:1, 0:1])

        # ---- pass 1: per-(b, c) raw sum / sum-of-squares ------------------
        # stat2[t][:, 0, b] = sum(x), stat2[t][:, 1, b] = sum(x^2)
        stat2 = [sb.tile([128, 2, BL], F32, tag=f"stat2_{t}", name=f"stat2_{t}") for t in range(CT)]
        sq_scr = sb.tile([128, HW], F32, tag="sq_scr")

        cc_in = [dram.tile([2 * BL, 128], F32, name=f"cc_in{t}") for t in range(CT)]
        cc_out = [dram.tile([NCORES, 2 * BL, 128], F32, name=f"cc_out{t}") for t in range(CT)]

        xtile = {}
        done_b = {t: 0 for t in range(CT)}
        s_bc = sb.tile([B, 2, CT, 128], F32, tag="s_bc")  # [64, (sum|sumsq), 2, 128]
        mu_bc = sb.tile([B, CT, 128], F32, tag="mu_bc")   # [64, 256] as [64, 2, 128]
        mu_cb = [
            sb.tile([128, B], F32, tag=f"mucb_{t}", name=f"mucb_{t}") for t in range(CT)
        ]
        g_ps = ps1.tile([B, B], F32, tag="g", name="g_ps")
        i64 = ident[:B, :B]

        for b, t in p1_order:
            i = (b, t)
            pool, tag = (res, "res") if i in idx_res else (xs, "xs")
            xt = pool.tile([128, HW], F32, tag=tag, name=f"xt_{b}_{t}")
            xtile[i] = xt
            nc.sync.dma_start(out=xt, in_=x_d[b, t])
            nc.vector.reduce_sum(out=stat2[t][:, 0, b : b + 1], in_=xt, axis=AX.X)
            nc.scalar.activation(
                out=sq_scr,
                in_=xt,
                func=AF.Square,
                accum_out=stat2[t][:, 1, b : b + 1],
            )
            done_b[t] += 1
            if done_b[t] < BL:
                continue

            # all 8 samples of channel-tile t done: transpose + AllGather it
            pt = ps.tile([2 * BL, 128], F32, tag="pt", name=f"pt_{t}")
            nc.tensor.transpose(pt, stat2[t].rearrange("p a q -> p (a q)"), ident)
            loc = sb.tile([2 * BL, 128], F32, tag="loc", name=f"loc_{t}")
            nc.scalar.copy(out=loc, in_=pt)
            nc.gpsimd.dma_start(out=cc_in[t], in_=loc)
            nc.gpsimd.collective_compute(
                "AllGather",
                ALU.bypass,
                replica_groups=[list(range(NCORES))],
                ins=[cc_in[t].opt()],
                outs=[cc_out[t].opt()],
            )
            # cc_out[t]: [rank, (m b_loc), p]; m=0 raw sum, m=1 raw sumsq
            v = cc_out[t].rearrange("r (m b) p -> m r b p", m=2)
            for m in range(2):
                nc.scalar.dma_start(out=s_bc[:, m, t, :], in_=v[m])
            # mean for this half + [c, b] layout + gram contribution
            nc.scalar.mul(out=mu_bc[:, t, :], in_=s_bc[:, 0, t, :], mul=1.0 / HW)
            pt2 = ps.tile([128, B], F32, tag="pt", name=f"pt2_{t}")
            nc.tensor.transpose(pt2, mu_bc[:, t, :], i64)
            nc.vector.tensor_copy(mu_cb[t], pt2)
            nc.tensor.matmul(
                g_ps, lhsT=mu_cb[t], rhs=mu_cb[t], start=(t == 0), stop=(t == CT - 1)
            )

        mu_bc2 = mu_bc.rearrange("q t p -> q (t p)")      # [64, 256] views
        s2_bc2 = s_bc[:, 1].rearrange("q t p -> q (t p)")

        # ---- FINCH: 1-NN graph + connected-component closure --------------
        dtmp = sb.tile([B, B], F32, tag="dtmp")
        nc.vector.tensor_mul(dtmp, g_ps, i64)
        dg = sb.tile([B, 1], F32, tag="dg")
        nc.vector.reduce_sum(out=dg, in_=dtmp, axis=AX.X)
        rdg0 = sb.tile([B, 1], F32, tag="rdg0")
        nc.vector.reciprocal(rdg0, dg)
        rdg = sb.tile([B, 1], F32, tag="rdg")
        nc.scalar.sqrt(rdg, rdg0)                         # 1/||mu_j||

        d_sb = sb.tile([B, B], F32, tag="d_sb")           # rows j scaled by rdg[j]
        nc.vector.tensor_scalar_mul(d_sb, g_ps, rdg)
        c_ps = ps.tile([B, B], F32, tag="pg", name="c_ps")
        nc.tensor.transpose(c_ps, d_sb, i64)              # C[i,j] = G[i,j]/||mu_j||
        c_m = sb.tile([B, B], F32, tag="c_m")
        nc.vector.scalar_tensor_tensor(
            out=c_m, in0=i64, scalar=NEG, in1=c_ps, op0=ALU.mult, op1=ALU.add
        )
        mx = sb.tile([B, 1], F32, tag="mx")
        nc.vector.reduce_max(out=mx, in_=c_m, axis=AX.X)
        p_sb = sb.tile([B, B], F32, tag="p_sb")           # one-hot nearest neighbor
        nc.vector.tensor_scalar(out=p_sb, in0=c_m, scalar1=mx, scalar2=None, op0=ALU.is_equal)

        pt_ps = ps.tile([B, B], F32, tag="pg", name="pt_ps")
        nc.tensor.transpose(pt_ps, p_sb, i64)
        pt_sb = sb.tile([B, B], F32, tag="pt_sb")
        nc.scalar.copy(out=pt_sb, in_=pt_ps)
        ppt_ps = ps.tile([B, B], F32, tag="pg", name="ppt_ps")
        nc.tensor.matmul(ppt_ps, lhsT=pt_sb, rhs=pt_sb)   # P @ P.T  (diag == 1)

        acc1 = sb.tile([B, B], F32, tag="acc1")
        nc.vector.tensor_add(acc1, p_sb, pt_sb)
        acc3 = sb.tile([B, B], F32, tag="acc3")
        nc.vector.scalar_tensor_tensor(
            out=acc3, in0=ppt_ps, scalar=1.0, in1=acc1, op0=ALU.mult, op1=ALU.add
        )
        r_cur = sb.tile([B, B], F32, tag="r0", name="r0")
        nc.vector.tensor_scalar(out=r_cur, in0=acc3, scalar1=0.5, scalar2=None, op0=ALU.is_ge)

        for it in range(6):                               # R^(2^6) covers paths <= 64
            s_ps = ps.tile([B, B], F32, tag="pg", name=f"s_ps{it}")
            nc.tensor.matmul(s_ps, lhsT=r_cur, rhs=r_cur)
            r_nxt = sb.tile([B, B], F32, tag=f"r{(it % 2) + 1}", name=f"r{it + 1}")
            nc.vector.tensor_scalar(out=r_nxt, in0=s_ps, scalar1=0.5, scalar2=None, op0=ALU.is_ge)
            r_cur = r_nxt

        # ---- cluster stats in matrix form ---------------------------------
        rowN = sb.tile([B, 1], F32, tag="rowN")
        nc.vector.reduce_sum(out=rowN, in_=r_cur, axis=AX.X)
        dE = sb.tile([B, 1], F32, tag="dE")
        nc.vector.tensor_scalar(out=dE, in0=rowN, scalar1=float(EPS), scalar2=None, op0=ALU.add)
        rinv = sb.tile([B, 1], F32, tag="rinv")
        nc.vector.reciprocal(rinv, dE)
        rinv2 = sb.tile([B, 1], F32, tag="rinv2")         # rinv / (HW - 1)
        nc.vector.tensor_scalar_mul(rinv2, rinv, 1.0 / (HW - 1.0))

        s1_ps = ps1.tile([B, C], F32, tag="s1p", name="s1_ps")
        nc.tensor.matmul(s1_ps, lhsT=r_cur, rhs=mu_bc2)
        mu_g = sb.tile([B, C], F32, tag="mu_g")
        nc.vector.tensor_scalar_mul(mu_g, s1_ps, rinv)

        # smu = sigma2 + mu^2 = (sum(x^2) - mu^2) / (HW - 1)
        # sig_g = (M @ (sum(x^2) - mu^2)) * rinv/(HW-1) - mu_g^2
        musq = sb.tile([B, C], F32, tag="musq")
        nc.vector.tensor_mul(musq, mu_bc2, mu_bc2)
        smu = sb.tile([B, C], F32, tag="smu")
        nc.vector.tensor_sub(smu, s2_bc2, musq)
        ss_ps = ps.tile([B, C], F32, tag="ssp", name="ss_ps")
        nc.tensor.matmul(ss_ps, lhsT=r_cur, rhs=smu)
        mgsq = sb.tile([B, C], F32, tag="musq", name="mgsq")
        nc.vector.tensor_mul(mgsq, mu_g, mu_g)
        sig_g = sb.tile([B, C], F32, tag="sig_g")
        nc.vector.scalar_tensor_tensor(
            out=sig_g, in0=ss_ps, scalar=rinv2, in1=mgsq, op0=ALU.mult, op1=ALU.subtract
        )

        # fused affine: out = A * x + Bc
        vV = sb.tile([B, C], F32, tag="vV")
        nc.vector.scalar_tensor_tensor(
            out=vV, in0=sig_g, scalar=float(rate_), in1=vb_sb, op0=ALU.mult, op1=ALU.add
        )
        vr = sb.tile([B, C], F32, tag="vr")
        nc.vector.reciprocal(vr, vV)
        rq = sb.tile([B, C], F32, tag="rq")
        nc.scalar.sqrt(rq, vr)                            # rsqrt(V)
        a_t = sb.tile([B, C], F32, tag="a_t")
        nc.vector.tensor_mul(a_t, rq, wt_sb)
        t4 = sb.tile([B, C], F32, tag="vV", name="t4")
        nc.vector.scalar_tensor_tensor(
            out=t4, in0=mu_g, scalar=float(rate_), in1=mb_sb, op0=ALU.mult, op1=ALU.add
        )
        t5 = sb.tile([B, C], F32, tag="rq", name="t5")
        nc.vector.tensor_mul(t5, t4, a_t)
        b_t = sb.tile([B, C], F32, tag="b_t")
        nc.vector.tensor_sub(b_t, bs_sb, t5)

        # select this core's 8 rows, transpose to [c, b_loc]
        asel_ps = ps.tile([BL, C], F32, tag="ssp", name="asel_ps")
        nc.tensor.matmul(asel_ps, lhsT=sel_sb, rhs=a_t)
        asel = sb.tile([BL, C], F32, tag="asel")
        nc.scalar.copy(out=asel, in_=asel_ps)
        bsel_ps = ps.tile([BL, C], F32, tag="ssp", name="bsel_ps")
        nc.tensor.matmul(bsel_ps, lhsT=sel_sb, rhs=b_t)
        bsel = sb.tile([BL, C], F32, tag="bsel")
        nc.scalar.copy(out=bsel, in_=bsel_ps)

        a_own, b_own = [], []
        for t in range(CT):
            ta_ps = ps.tile([128, BL], F32, tag="pt", name=f"ta_{t}")
            nc.tensor.transpose(ta_ps, asel[:, t * 128 : (t + 1) * 128], ident[:BL, :BL])
            ao = sb.tile([128, BL], F32, tag=f"aown_{t}", name=f"aown_{t}")
            nc.scalar.copy(out=ao, in_=ta_ps)
            a_own.append(ao)
            tb_ps = ps.tile([128, BL], F32, tag="pt", name=f"tb_{t}")
            nc.tensor.transpose(tb_ps, bsel[:, t * 128 : (t + 1) * 128], ident[:BL, :BL])
            bo = sb.tile([128, BL], F32, tag=f"bown_{t}", name=f"bown_{t}")
            nc.scalar.copy(out=bo, in_=tb_ps)
            b_own.append(bo)

        # ---- pass 2: fused normalize, in place, stores on the ACT queue ---
        # streamed first: their prefetched buffers recycle earliest
        for i in idx_stream:
            b, t = i
            xt2 = xs.tile([128, HW], F32, tag="xs", name=f"xt2_{b}_{t}")
            nc.sync.dma_start(out=xt2, in_=x_d[b, t])
            nc.scalar.activation(
                out=xt2,
                in_=xt2,
                func=AF.Identity,
                bias=b_own[t][:, b : b + 1],
                scale=a_own[t][:, b : b + 1],
            )
            nc.scalar.dma_start(out=out_d[b, t], in_=xt2)
        for i in idx_res:
            b, t = i
            xt = xtile[i]
            nc.scalar.activation(
                out=xt,
                in_=xt,
                func=AF.Identity,
                bias=b_own[t][:, b : b + 1],
                scale=a_own[t][:, b : b + 1],
            )
            nc.scalar.dma_start(out=out_d[b, t], in_=xt)

    nc.compile()
    nc.m = get_hw_module(nc.m)
    return nc


_CACHE: dict = {}


def _prepare(x, running_mean, running_var, weight, bias, source_rate):
    x = np.ascontiguousarray(np.asarray(x, dtype=np.float32))
    rm = np.asarray(running_mean, np.float32)
    rv = np.asarray(running_var, np.float32)
    wt = np.asarray(weight, np.float32)
    bs = np.asarray(bias, np.float32)
    sr = np.float32(min(max(float(np.asarray(source_rate)), 0.0), 1.0))
    rate_ = float(np.float32(1.0) - sr)

    vb = (sr * rv + np.float32(EPS)).astype(np.float32)
    mb = (sr * rm).astype(np.float32)
    vb_bc = np.ascontiguousarray(np.broadcast_to(vb, (B, C)))
    mb_bc = np.ascontiguousarray(np.broadcast_to(mb, (B, C)))
    wt_bc = np.ascontiguousarray(np.broadcast_to(wt, (B, C)))
    bs_bc = np.ascontiguousarray(np.broadcast_to(bs, (B, C)))
    ident = np.eye(128, dtype=np.float32)

    in_maps = []
    for k in range(NCORES):
        sel = np.zeros((B, BL), np.float32)
        sel[k * BL + np.arange(BL), np.arange(BL)] = 1.0
        in_maps.append(
            {
                "x": x[k * BL : (k + 1) * BL].reshape(BL, CT, 128, HW),
                "vb": vb_bc,
                "mb": mb_bc,
                "wt": wt_bc,
                "bs": bs_bc,
                "sel": sel,
                "ident": ident,
            }
        )
    return rate_, in_maps


def run(inputs: dict, trace: bool = False, **kw):
    rate_, in_maps = _prepare(**inputs)
    if rate_ not in _CACHE:
        _CACHE[rate_] = build_program(rate_)
    nc = _CACHE[rate_]
    res = bass_utils.run_bass_kernel_spmd(
        nc, in_maps, core_ids=list(range(NCORES)), trace=trace, **kw
    )
    outs = [np.asarray(r["out"]).reshape(BL, C, H, W) for r in res.results]
    return np.concatenate(outs, axis=0), res


def kernel(**inputs) -> np.ndarray:
    out, _ = run(inputs)
    return out
